# revision 27
# baseline (speedup 1.0000x reference)
# Trainium2 Bass kernel for a LIIF-style implicit image network.
#
# Model (per batch element): EDSR-lite conv encoder (head 3->64 + 2 residual
# blocks) -> coef/freq 64->256 convs -> nearest-neighbor grid sample at Q
# query points -> per-query cos/sin feature modulation -> 5-layer MLP -> rgb.
#
# Distribution: pure data parallel, one image (+ its queries) per NeuronCore.
#
# On-core dataflow (all matmuls bf16 with fp32 PSUM accumulation):
#   * feature maps live in SBUF as [128, 130*130] zero-padded buffers whose
#     partitions 0:64 hold the map and 64:128 hold the map shifted one column
#     left; 3x3 convs then become 3 K=128 + 3 K=64 PSUM-accumulated matmuls
#     per 512-pixel chunk (tap-pair packing).
#   * qf = cos/sin(pi*freq) computed per PIXEL (Q == H*W) with an exact
#     mod-2 range reduction (round-half-even via +-2^23 on ACT) + Sin LUT.
#   * z = (coef*qf)^T @ W0[:256] is produced pixel-major straight from the
#     TensorEngine (g is the stationary operand) and staged to a DRAM table.
#   * the grid-sample gather is a dma_gather(transpose=True): 512B bf16 rows
#     fetched by int16 index, landing as [128, 2, nq] = channels-on-partitions.
#   * MLP runs in 512-query chunks; (tanh(x)+1)/2 == sigmoid(2x) is one ACT op.

import numpy as np
import ml_dtypes

import concourse.bass as bass
import concourse.tile as tile
import concourse.mybir as mybir
from concourse import bacc
from concourse.bass_utils import run_bass_kernel_spmd

BF = mybir.dt.bfloat16
F32 = mybir.dt.float32
I16 = mybir.dt.int16
AF = mybir.ActivationFunctionType
OP = mybir.AluOpType

B, Q, H, W = 8, 16384, 128, 128
C = 64
HID = 256
HP = WP = 130          # padded map dims
NPAD = HP * WP         # 16900
NPIX = H * W           # 16384
RCH = 4                # rows per conv chunk
PCH = RCH * W          # 512 pixels per conv chunk
NCH = NPIX // PCH      # 32 chunks
QCH = 512              # queries per MLP chunk
QBLK = 4096            # queries per gather block
TWO23 = float(3 * 2 ** 22)  # 1.5*2^23: round-to-int magic constant valid for |x| < 2^22
PI = float(np.pi)

bfloat16 = ml_dtypes.bfloat16


# --------------------------------------------------------------------------
# graph builder
# --------------------------------------------------------------------------

def _pad_view(t, p0, pn, r0, nr, c0, ncols):
    """[pn, nr, ncols] view into a padded [*, HP*WP] map tile."""
    return t[p0:p0 + pn, :].rearrange("p (r c) -> p r c", c=WP)[:, r0:r0 + nr, c0:c0 + ncols]


def build_nc(debug=False, stage='full'):
    nc = bacc.Bacc("TRN2", target_bir_lowering=False, debug=False)

    # ---- DRAM parameters (per-core shapes) ----
    inp_d = nc.dram_tensor("inp", [3, NPIX], F32, kind="ExternalInput")
    coord_d = nc.dram_tensor("coord", [Q, 2], F32, kind="ExternalInput")
    cell_d = nc.dram_tensor("cell", [1, Q], F32, kind="ExternalInput")

    hw_d = nc.dram_tensor("hw", [27, C], BF, kind="ExternalInput")
    hb_d = nc.dram_tensor("hb", [C, 1], F32, kind="ExternalInput")
    rbw_d = nc.dram_tensor("rbw", [128, 4 * 6 * C], BF, kind="ExternalInput")
    rbb_d = nc.dram_tensor("rbb", [C, 4], F32, kind="ExternalInput")
    cfw_d = nc.dram_tensor("cfw", [128, 2 * 6 * HID], BF, kind="ExternalInput")
    coefb_d = nc.dram_tensor("coefb", [128, 2], F32, kind="ExternalInput")
    fc0_d = nc.dram_tensor("fc0", [128, 2], F32, kind="ExternalInput")
    w0_d = nc.dram_tensor("w0", [128, 2 * HID], BF, kind="ExternalInput")
    w0c_d = nc.dram_tensor("w0c", [128, 2], F32, kind="ExternalInput")
    b0_d = nc.dram_tensor("b0", [128, 2], F32, kind="ExternalInput")
    wm_d = nc.dram_tensor("wm", [128, 12 * 128], BF, kind="ExternalInput")
    bm_d = nc.dram_tensor("bm", [128, 6], F32, kind="ExternalInput")
    w4_d = nc.dram_tensor("w4", [128, 6], BF, kind="ExternalInput")
    b4_d = nc.dram_tensor("b4", [3, 1], F32, kind="ExternalInput")
    ident_d = nc.dram_tensor("ident", [128, 128], BF, kind="ExternalInput")

    out_d = nc.dram_tensor("out", [3, Q], F32, kind="ExternalOutput")
    if debug:
        dbg_x0 = nc.dram_tensor("dbg_x0", [128, NPAD], F32, kind="ExternalOutput")
        dbg_x2 = nc.dram_tensor("dbg_x2", [128, NPAD], F32, kind="ExternalOutput")
        dbg_g = nc.dram_tensor("dbg_g", [128, 2, 512], F32, kind="ExternalOutput")
        dbg_qf = nc.dram_tensor("dbg_qf", [128, 2, 512], F32, kind="ExternalOutput")
        dbg_fp = nc.dram_tensor("dbg_fp", [128, 2, 512], F32, kind="ExternalOutput")
        dbg_cp = nc.dram_tensor("dbg_cp", [128, 2, 512], F32, kind="ExternalOutput")
        dbg_z = nc.dram_tensor("dbg_z", [NPIX, HID], BF, kind="ExternalOutput")
        dbg_gth = nc.dram_tensor("dbg_gth", [128, 4, HID], BF, kind="ExternalOutput")

    from contextlib import ExitStack
    with tile.TileContext(nc) as tc, ExitStack() as ctx:
        const = ctx.enter_context(tc.tile_pool(name="const", bufs=1))
        work = ctx.enter_context(tc.tile_pool(name="work", bufs=3))
        ps = ctx.enter_context(tc.tile_pool(name="ps", bufs=6, space="PSUM"))
        ps4 = ctx.enter_context(tc.tile_pool(name="ps4", bufs=2, space="PSUM"))
        dram = ctx.enter_context(tc.tile_pool(name="dram", bufs=1, space="DRAM"))

        # ---- constants into SBUF ----
        def cload(name, shape, dt, src):
            t = const.tile(shape, dt, tag=name)
            nc.sync.dma_start(t[:], src)
            return t

        hw_sb = cload("hw", [27, C], BF, hw_d.ap())
        hb_sb = cload("hb", [C, 1], F32, hb_d.ap())
        rbw_sb = cload("rbw", [128, 4 * 6 * C], BF, rbw_d.ap())
        rbb_sb = cload("rbb", [C, 4], F32, rbb_d.ap())
        cfw_sb = cload("cfw", [128, 2 * 6 * HID], BF, cfw_d.ap())
        coefb_sb = cload("coefb", [128, 2], F32, coefb_d.ap())
        fc0_sb = cload("fc0", [128, 2], F32, fc0_d.ap())
        w0_sb = cload("w0", [128, 2 * HID], BF, w0_d.ap())
        w0c_sb = cload("w0c", [128, 2], F32, w0c_d.ap())
        b0_sb = cload("b0", [128, 2], F32, b0_d.ap())
        wm_sb = cload("wm", [128, 12 * 128], BF, wm_d.ap())
        bm_sb = cload("bm", [128, 6], F32, bm_d.ap())
        w4_sb = cload("w4", [128, 6], BF, w4_d.ap())
        b4_sb = cload("b4", [3, 1], F32, b4_d.ap())
        ident_sb = cload("ident", [128, 128], BF, ident_d.ap())

        tp23 = const.tile([128, 1], F32, tag="tp23")
        nc.vector.memset(tp23[:], TWO23)
        tn23 = const.tile([128, 1], F32, tag="tn23")
        nc.vector.memset(tn23[:], -TWO23)

        idx32 = const.tile([128, 128], mybir.dt.int32, tag="idx32")
        ones1 = const.tile([1, 128], BF, tag="ones1")
        nc.vector.memset(ones1[:], 1.0)

        with tc.tile_pool(name="prep", bufs=1) as prep:
            # query q = c*128 + p lives at [partition p, col c]
            coord_sb = prep.tile([128, 256], F32, tag="coord")
            nc.sync.dma_start(coord_sb[:].rearrange("p (c k) -> p c k", k=2),
                              coord_d.ap().rearrange("(c p) k -> p c k", p=128))

            # query indices: iy*128+ix with round-half-even
            ctmp = prep.tile([128, 256], F32, tag="ctmp")
            nc.vector.tensor_scalar(ctmp[:], coord_sb[:], 1.0, 128.0, OP.add, OP.mult)
            nc.vector.tensor_scalar(ctmp[:], ctmp[:], -1.0, 0.5, OP.add, OP.mult)
            crnd = prep.tile([128, 256], F32, tag="crnd")
            nc.vector.tensor_scalar(crnd[:], ctmp[:], TWO23, None, OP.add)
            nc.vector.tensor_scalar(crnd[:], crnd[:], TWO23, None, OP.subtract)
            nc.vector.tensor_scalar(crnd[:], crnd[:], 0.0, 127.0, OP.max, OP.min)
            cr3 = crnd[:].rearrange("p (c k) -> p c k", k=2)
            idxf = prep.tile([128, 128], F32, tag="idxf")
            nc.vector.scalar_tensor_tensor(idxf[:], cr3[:, :, 0], float(W),
                                           cr3[:, :, 1], OP.mult, OP.add)
            nc.vector.tensor_copy(idx32[:], idxf[:])


        # ---- conv encoder ----
        maps_cm = tc.tile_pool(name="maps", bufs=3)
        maps = maps_cm.__enter__()

        def new_map(full_zero=False):
            t = maps.tile([128, NPAD], BF, tag="map")
            if full_zero:
                nc.vector.memset(t[:], 0.0)
            else:
                v = t[:].rearrange("p (r c) -> p r c", c=WP)
                nc.vector.memset(v[:, 0, :], 0.0)        # top pad row
                nc.vector.memset(v[:, HP - 1, :], 0.0)   # bottom pad row
                nc.vector.memset(v[:, :, 0:1], 0.0)      # left pad col
                nc.vector.memset(v[:, :, 128:130], 0.0)  # right pad cols
            return t

        # head im2col buffer: partition 3*t + c holds channel c shifted by -tap t
        head_t = new_map(full_zero=True)
        taps = [(dy, dx) for dy in (-1, 0, 1) for dx in (-1, 0, 1)]
        inp3 = inp_d.ap().rearrange("c (h w) -> c h w", w=W)
        for ti, (dy, dx) in enumerate(taps):
            dst = _pad_view(head_t, 3 * ti, 3, 1 - dy, H, 1 - dx, W)
            nc.gpsimd.dma_start(dst, inp3)  # f32 -> bf16 cast

        def conv_chunks(src_t, lhs_cols, lhs_sb, m_out, out_cb):
            """One 3x3 conv via tap-pair packing.

            lhs_cols(g) -> column slice of lhs_sb for group g (width m_out).
            out_cb(ci, psum_ap) consumes the accumulated [m_out, RCH, W] chunk.
            """
            for ci in range(NCH):
                y0 = ci * RCH
                pt = ps.tile([m_out, 512], F32, tag="ps")
                pv = pt[:].rearrange("p (r c) -> p r c", c=W)
                for g in range(6):
                    dy = (g % 3) - 1
                    if g < 3:   # pair group: taps (dy,-1) on low, (dy,0) on high
                        rhs = _pad_view(src_t, 0, 128, 1 + y0 + dy, RCH, 0, W)
                        lhs = lhs_sb[:, lhs_cols(g)]
                    else:       # single group: tap (dy,+1) via the high copy
                        rhs = _pad_view(src_t, 64, 64, 1 + y0 + dy, RCH, 1, W)
                        lhs = lhs_sb[64:128, lhs_cols(g)]
                    nc.tensor.matmul(pv, lhs, rhs, start=(g == 0), stop=(g == 5))
                out_cb(ci, pv)

        def evict_pair(dst_t, ci, psum_ap, mode, bias_ap, res_t=None):
            """Write psum chunk to dst low view + the col-shifted high view."""
            y0 = ci * RCH
            lo = _pad_view(dst_t, 0, C, 1 + y0, RCH, 1, W)
            hi = _pad_view(dst_t, C, C, 1 + y0, RCH, 0, W)
            if mode == "relu":
                nc.scalar.activation(lo, psum_ap, AF.Relu, bias=bias_ap)
                nc.vector.tensor_scalar(hi, psum_ap, bias_ap, 0.0, OP.add, OP.max)
            elif mode == "copy":
                nc.scalar.activation(lo, psum_ap, AF.Identity, bias=bias_ap)
                nc.vector.tensor_scalar(hi, psum_ap, bias_ap, None, OP.add)
            else:  # residual
                res_lo = _pad_view(res_t, 0, C, 1 + y0, RCH, 1, W)
                nc.vector.scalar_tensor_tensor(lo, psum_ap, bias_ap, res_lo,
                                               OP.add, OP.add)
                nc.vector.scalar_tensor_tensor(hi, psum_ap, bias_ap, res_lo,
                                               OP.add, OP.add)

        # head conv -> x0
        x0_t = new_map()
        for ci in range(NCH):
            y0 = ci * RCH
            pt = ps.tile([C, 512], F32, tag="ps")
            pv = pt[:].rearrange("p (r c) -> p r c", c=W)
            rhs = _pad_view(head_t, 0, 27, 1 + y0, RCH, 1, W)
            nc.tensor.matmul(pv, hw_sb[:], rhs, start=True, stop=True)
            evict_pair(x0_t, ci, pv, "copy", hb_sb[:])

        # residual blocks
        def rb_cols(conv_i):
            return lambda g: bass.ts(conv_i * 6 + g, C)

        m1_t = new_map()
        conv_chunks(x0_t, rb_cols(0), rbw_sb, C,
                    lambda ci, pv: evict_pair(m1_t, ci, pv, "relu", rbb_sb[:, 0:1]))
        x1_t = new_map()
        conv_chunks(m1_t, rb_cols(1), rbw_sb, C,
                    lambda ci, pv: evict_pair(x1_t, ci, pv, "res", rbb_sb[:, 1:2], x0_t))
        m2_t = new_map()
        conv_chunks(x1_t, rb_cols(2), rbw_sb, C,
                    lambda ci, pv: evict_pair(m2_t, ci, pv, "relu", rbb_sb[:, 2:3]))
        x2_t = new_map()
        conv_chunks(m2_t, rb_cols(3), rbw_sb, C,
                    lambda ci, pv: evict_pair(x2_t, ci, pv, "res", rbb_sb[:, 3:4], x1_t))

        if debug:
            nc.gpsimd.dma_start(dbg_x0.ap(), x0_t[:])  # bf16 -> f32 cast
            nc.gpsimd.dma_start(dbg_x2.ap(), x2_t[:])

        if stage == 'conv':
            nc.gpsimd.dma_start(out_d.ap()[:, 0:Q], x2_t[0:3, 0:Q])
            maps_cm.__exit__(None, None, None)
            nc.compile_marker = None  # noqa
        # ---- coef/freq convs + qf + g + z (per pixel chunk) ----
        z_dram = dram.tile([NPIX, HID], BF, tag="zdram")

        def cf_lhs(conv_i, g, mt):
            base = (conv_i * 6 + g) * HID + mt * 128
            return cfw_sb[:, base:base + 128] if g < 3 else \
                cfw_sb[64:128, base:base + 128]

        for ci in range(NCH if stage != 'conv' else 0):
            y0 = ci * RCH
            gt = work.tile([128, 2, 512], BF, tag="g")
            for mt in range(2):
                # freq conv (permuted: mt0 -> cos channels, mt1 -> sin channels)
                fp = ps.tile([128, 512], F32, tag="ps")
                fpv = fp[:].rearrange("p (r c) -> p r c", c=W)
                for g in range(6):
                    dy = (g % 3) - 1
                    if g < 3:
                        rhs = _pad_view(x2_t, 0, 128, 1 + y0 + dy, RCH, 0, W)
                    else:
                        rhs = _pad_view(x2_t, 64, 64, 1 + y0 + dy, RCH, 1, W)
                    nc.tensor.matmul(fpv, cf_lhs(1, g, mt), rhs,
                                     start=(g == 0), stop=(g == 5))
                # range-reduce to [-0.5, 0.5] then Sin(2*pi*r)
                t1 = work.tile([128, 512], F32, tag="t1")
                nc.vector.tensor_scalar(t1[:], fp[:], 0.5, fc0_sb[:, mt:mt + 1],
                                        OP.mult, OP.add)
                t2 = work.tile([128, 512], F32, tag="t2")
                nc.scalar.activation(t2[:], t1[:], AF.Identity, bias=tp23[:])
                nc.scalar.activation(t2[:], t2[:], AF.Identity, bias=tn23[:])
                rr = work.tile([128, 512], F32, tag="rr")
                nc.vector.scalar_tensor_tensor(rr[:], t2[:], -1.0, t1[:],
                                               OP.mult, OP.add)
                qf = work.tile([128, 512], BF, tag="qf")
                nc.scalar.activation(qf[:], rr[:], AF.Sin, scale=2.0 * PI)
                if debug and ci == 0:
                    nc.gpsimd.dma_start(dbg_qf.ap()[:, mt, :], qf[:])
                    dfp = work.tile([128, 512], F32, tag="t1")
                    nc.vector.tensor_copy(dfp[:], fp[:])
                    nc.gpsimd.dma_start(dbg_fp.ap()[:, mt, :], dfp[:])

                # coef conv -> g = (coef + b) * qf
                cp = ps.tile([128, 512], F32, tag="ps")
                cpv = cp[:].rearrange("p (r c) -> p r c", c=W)
                for g in range(6):
                    dy = (g % 3) - 1
                    if g < 3:
                        rhs = _pad_view(x2_t, 0, 128, 1 + y0 + dy, RCH, 0, W)
                    else:
                        rhs = _pad_view(x2_t, 64, 64, 1 + y0 + dy, RCH, 1, W)
                    nc.tensor.matmul(cpv, cf_lhs(0, g, mt), rhs,
                                     start=(g == 0), stop=(g == 5))
                nc.vector.scalar_tensor_tensor(gt[:, mt, :], cp[:],
                                               coefb_sb[:, mt:mt + 1], qf[:],
                                               OP.add, OP.mult)
                if debug and ci == 0:
                    dcp = work.tile([128, 512], F32, tag="t1")
                    nc.vector.tensor_copy(dcp[:], cp[:])
                    nc.gpsimd.dma_start(dbg_cp.ap()[:, mt, :], dcp[:])

            # z chunk: [512 px, 256] pixel-major via g-stationary matmuls
            zsb = work.tile([128, RCH, HID], BF, tag="zsb")
            for s in range(RCH):
                zp = ps.tile([128, HID], F32, tag="ps")
                for kt in range(2):
                    nc.tensor.matmul(zp[:], gt[:, kt, s * 128:(s + 1) * 128],
                                     w0_sb[:, bass.ts(kt, HID)],
                                     start=(kt == 0), stop=(kt == 1))
                nc.vector.tensor_copy(zsb[:, s, :], zp[:])
            zdst = z_dram[ci * PCH:(ci + 1) * PCH, :].rearrange(
                "(s p) n -> p s n", s=RCH)
            nc.sync.dma_start(zdst, zsb[:])
            if debug and ci == 0:
                nc.gpsimd.dma_start(dbg_g.ap(), gt[:])

        # ---- gather + MLP ----
        if stage == 'z':
            nc.gpsimd.dma_start(out_d.ap()[:, 0:HID], z_dram[0:3, :])
        if stage != 'conv':
            maps_cm.__exit__(None, None, None)
        for ci in range(NCH if stage in ('full', 'gth', 'gtt') else 0):
            qs = slice(ci * QCH, (ci + 1) * QCH)
            # gather 512 query rows (query-major), then PE-transpose each
            # 128x128 block to channels-on-partitions
            qg = work.tile([128, 4, HID], BF, tag="qg")
            for s in range(4):
                nc.gpsimd.indirect_dma_start(
                    qg[:, s, :], None, z_dram[:, :],
                    bass.IndirectOffsetOnAxis(
                        ap=idx32[:, 4 * ci + s:4 * ci + s + 1], axis=0))
            if debug and ci == 0:
                nc.sync.dma_start(dbg_gth.ap(), qg[:])
            if stage == 'gth':
                if ci == 0:
                    nc.gpsimd.dma_start(
                        out_d.ap()[:, 0:1024],
                        qg[0:3, :, :].rearrange("p a b -> p (a b)"))
                continue
            zgt = work.tile([128, 2, QCH], BF, tag="zg")
            for s in range(4):
                for mt in range(2):
                    tp = ps.tile([128, 128], BF, tag="ps")
                    nc.tensor.transpose(tp[:], qg[:, s, mt * 128:(mt + 1) * 128],
                                        ident_sb[:])
                    nc.vector.tensor_copy(zgt[:, mt, s * 128:(s + 1) * 128], tp[:])
            if stage == 'gtt':
                if ci == 0:
                    nc.gpsimd.dma_start(
                        out_d.ap()[:, 0:1024],
                        zgt[0:3, :, :].rearrange("p a b -> p (a b)"))
                continue
            csb = work.tile([1, QCH], BF, tag="csb")
            nc.gpsimd.dma_start(csb[:], cell_d.ap()[:, qs])  # f32->bf16 cast
            cb_ps = ps.tile([128, QCH], F32, tag="ps")
            nc.tensor.matmul(cb_ps[:], ones1[:], csb[:], start=True, stop=True)
            h = work.tile([128, 2, QCH], BF, tag="h_a")
            for mt in range(2):
                h0f = work.tile([128, QCH], F32, tag="h0f")
                nc.vector.scalar_tensor_tensor(
                    h0f[:], cb_ps[:], w0c_sb[:, mt:mt + 1],
                    zgt[:, mt, :], OP.mult, OP.add)
                nc.scalar.activation(h[:, mt, :], h0f[:], AF.Relu,
                                     bias=b0_sb[:, mt:mt + 1])
            for l in range(3):
                hn = work.tile([128, 2, QCH], BF,
                               tag=("h_b" if l % 2 == 0 else "h_a"))
                for mt in range(2):
                    pp = ps.tile([128, QCH], F32, tag="ps")
                    for kt in range(2):
                        nc.tensor.matmul(
                            pp[:],
                            wm_sb[:, bass.ts(l * 4 + kt * 2 + mt, 128)],
                            h[:, kt, :], start=(kt == 0), stop=(kt == 1))
                    if mt == 0:
                        nc.scalar.activation(hn[:, mt, :], pp[:], AF.Relu,
                                             bias=bm_sb[:, 2 * l:2 * l + 1])
                    else:
                        nc.vector.tensor_scalar(hn[:, mt, :], pp[:],
                                                bm_sb[:, 2 * l + 1:2 * l + 2],
                                                0.0, OP.add, OP.max)
                h = hn
            p4 = ps4.tile([3, QCH], F32, tag="ps4")
            for kt in range(2):
                nc.tensor.matmul(p4[:], w4_sb[:, bass.ts(kt, 3)], h[:, kt, :],
                                 start=(kt == 0), stop=(kt == 1))
            # (tanh(x + b4) + 1) / 2 == sigmoid(2x + 2*b4)
            osb = work.tile([3, QCH], F32, tag="osb")
            nc.scalar.activation(osb[:], p4[:], AF.Sigmoid,
                                 bias=b4_sb[:], scale=2.0)
            nc.sync.dma_start(out_d.ap()[:, qs], osb[:])

        if debug:
            nc.sync.dma_start(dbg_z.ap(), z_dram[:, :])

    nc.compile()
    return nc


# --------------------------------------------------------------------------
# host-side weight prep
# --------------------------------------------------------------------------

def _prep_weights(params):
    p = {k: np.asarray(v, np.float32) for k, v in params.items()}
    bf16 = lambda x: np.ascontiguousarray(x).astype(bfloat16)
    f32 = lambda x: np.ascontiguousarray(x).astype(np.float32)

    taps = [(dy, dx) for dy in (-1, 0, 1) for dx in (-1, 0, 1)]
    hw = np.zeros((27, C), np.float32)
    for ti, (dy, dx) in enumerate(taps):
        for c_ in range(3):
            hw[3 * ti + c_] = p["head_w"][:, c_, dy + 1, dx + 1]

    def pack6(w):  # w: [O, I, 3, 3] -> [6, 128, O]
        O = w.shape[0]
        g6 = np.zeros((6, 128, O), np.float32)
        for gi, dy in enumerate((-1, 0, 1)):
            g6[gi, 0:C] = w[:, :, dy + 1, 0].T      # tap (dy, -1) on low copy
            g6[gi, C:128] = w[:, :, dy + 1, 1].T    # tap (dy, 0) on high copy
            g6[3 + gi, C:128] = w[:, :, dy + 1, 2].T  # tap (dy, +1), single
        return g6

    rbw = np.stack([pack6(p["rb1_w1"]), pack6(p["rb1_w2"]),
                    pack6(p["rb2_w1"]), pack6(p["rb2_w2"])])
    rbb = np.stack([p["rb1_b1"], p["rb1_b2"], p["rb2_b1"], p["rb2_b2"]])[..., None]

    perm = np.concatenate([np.arange(0, HID, 2), np.arange(1, HID, 2)])
    freq_w_p = p["freq_w"][perm]
    freq_b_p = p["freq_b"][perm]
    cfw = np.stack([pack6(p["coef_w"]), pack6(freq_w_p)])
    coefb = p["coef_b"].reshape(2, 128, 1)
    fc0 = np.stack([(freq_b_p[:128] + 0.5) / 2.0,
                    freq_b_p[128:] / 2.0]).reshape(2, 128, 1)

    w0 = p["mlp_w0"][:HID].reshape(2, 128, HID)
    w0c = p["mlp_w0"][HID].reshape(2, 128, 1)
    b0 = p["mlp_b0"].reshape(2, 128, 1)
    # wm[l, kt, mt] = W[l][kt*128:(kt+1)*128, mt*128:(mt+1)*128]
    wm2 = np.zeros((3, 2, 2, 128, 128), np.float32)
    for l in range(3):
        Wl = [p["mlp_w1"], p["mlp_w2"], p["mlp_w3"]][l]
        for kt in range(2):
            for mt in range(2):
                wm2[l, kt, mt] = Wl[kt * 128:(kt + 1) * 128, mt * 128:(mt + 1) * 128]
    bm = np.stack([p["mlp_b1"], p["mlp_b2"], p["mlp_b3"]]).reshape(3, 2, 128, 1)
    w4 = p["mlp_w4"].reshape(2, 128, 3)
    b4 = p["mlp_b4"].reshape(3, 1)

    return {
        "hw": bf16(hw), "hb": f32(p["head_b"].reshape(C, 1)),
        "rbw": bf16(rbw.transpose(2, 0, 1, 3).reshape(128, -1)),
        "rbb": f32(rbb[:, :, 0].T),
        "cfw": bf16(cfw.transpose(2, 0, 1, 3).reshape(128, -1)),
        "coefb": f32(coefb[:, :, 0].T), "fc0": f32(fc0[:, :, 0].T),
        "w0": bf16(w0.transpose(1, 0, 2).reshape(128, -1)),
        "w0c": f32(w0c[:, :, 0].T), "b0": f32(b0[:, :, 0].T),
        "wm": bf16(wm2.transpose(3, 0, 1, 2, 4).reshape(128, -1)),
        "bm": f32(bm[:, :, :, 0].transpose(2, 0, 1).reshape(128, -1)),
        "w4": bf16(w4.transpose(1, 0, 2).reshape(128, -1)),
        "b4": f32(b4),
        "ident": bf16(np.eye(128, dtype=np.float32)),
    }


def make_in_maps(inp, coord, cell, params):
    wts = _prep_weights(params)
    inp = np.asarray(inp, np.float32)
    coord = np.asarray(coord, np.float32)
    cell = np.asarray(cell, np.float32)
    in_maps = []
    for i in range(B):
        m = dict(wts)
        m["inp"] = np.ascontiguousarray(inp[i].reshape(3, NPIX))
        m["coord"] = np.ascontiguousarray(coord[i])
        m["cell"] = np.ascontiguousarray(cell[i].reshape(1, Q))
        in_maps.append(m)
    return in_maps


_NC_CACHE = {}


def kernel(inp, coord, cell, params):
    if "nc" not in _NC_CACHE:
        _NC_CACHE["nc"] = build_nc()
    nc = _NC_CACHE["nc"]
    in_maps = make_in_maps(inp, coord, cell, params)
    res = run_bass_kernel_spmd(nc, in_maps, core_ids=list(range(B)))
    out = np.stack([res.results[i]["out"] for i in range(B)])  # [B, 3, Q]
    return np.ascontiguousarray(out.transpose(0, 2, 1)).astype(np.float32)
